# revision 31
# baseline (speedup 1.0000x reference)
"""ColBERTer forward as a Trainium2 Bass/Tile kernel, data-parallel over 8 cores.

Problem shapes (hardcoded): B=128, LQ=32, LD=512, H=768, C=128.

Strategy (v2: bf16 datapath)
----------------------------
Pure data parallel: batch dim sharded 16-per-core across 8 NeuronCores.
Host-side prep casts doc/query hidden states and W_comp to bf16 and re-lays
them out H-partitioned so the device needs ZERO on-chip transposes:

  docp[core][p, (b*6+ht)*512 + l] = doc_hidden[core*16+b, l, ht*128+p]   (bf16)
  qtp [core][p, ht*512 + b*32+q] = query_hidden[core*16+b, q, ht*128+p]  (bf16)

bf16 matmuls run at 1 PE cycle/row (fp32 needs 2 half-speed passes = 4x),
and the dominant doc DMA halves to ~12.6 MB/core, so the kernel sits at the
HBM roofline (target_regime=memory).

Per batch on device:
  d_tokT[c, l] = sum_ht W_tile[ht].T @ docT_tile[ht]   (6 accumulating matmuls)
  D = bf16(d_tokT + b_comp)                            (ACT copy w/ bias, bf16 out)
  psum_s[:, kt*33:+33] = D_kt.T @ [qv(b) | w_stop]     (4 matmuls: scores + imp col)
  imp[l]  = relu(imp_col + b_stop)                     (ACT, 4 cols)
  imm     = imp * dm                                   (DVE, 4 cols)
  sm      = psum_s * imm + (dm-1)*1000                 (DVE tensor_scalar dual op x4)
  term[q] = max over the 4 k-tiles                     (DVE max tree)

q_vecs for all 16 batches are computed once per core (bf16, bias folded in).
The qm masking of q_vecs is algebraically redundant (masked q rows are
dropped by the final where(qm,...) sum), so it is skipped on device.

Host-side epilogue: per-(b,q) max over the 128 partitions, cls score (exact
fp32 dot of CLS rows), qm-masked sum of term, sigmoid(score_merger) merge.
"""

import numpy as np
from contextlib import ExitStack

import concourse.bass as bass
import concourse.tile as tile
from concourse import mybir
from concourse import bass_utils

F32 = mybir.dt.float32
BF16 = mybir.dt.bfloat16
NP_BF16 = mybir.dt.np(BF16)
AF = mybir.ActivationFunctionType
ALU = mybir.AluOpType

B, LQ, LD, H, C = 128, 32, 512, 768, 128
NCORES = 8
BPC = B // NCORES       # 16 batches per core
HT = H // 128           # 6 h-tiles
KT = LD // 128          # 4 doc-token tiles
CHUNK = 2               # max batches per doc DMA chunk
CHUNK_SIZES = [2, 2, 2, 2, 2, 2, 2, 2]
EW = LQ + 1             # fused rhs width: 32 qv cols + 1 w_stop col

# cons16: bf16 consolidated W + q^T tensor (single DMA lane for all matmuls)
CONS_W = 0                       # [0, 768): W_comp as [hp, ht, c]
CONS_QT = HT * 128               # [768, 3840): q^T as [hp, ht, b*32+q]
CONS16_COLS = CONS_QT + HT * 512

# aux32: f32 small-constants tensor (single DMA lane for ACT/DVE operands)
A_BCOMP = 0
A_BSTOP = 1
A_WST16 = 2                      # w_stop replicated x16 (qvw col fill)
A_DMK = A_WST16 + BPC            # dm arranged [kp, b*4+kt]
A_OFFK = A_DMK + BPC * KT        # (dm-1)*1000
AUX_COLS = A_OFFK + BPC * KT

_CACHE = {}


# engine -> its own semaphore-name prefix (strict-FIFO compute queues only;
# a wait on the engine's OWN completion sem is an ordering no-op on these).
_OWN_SEM_PREFIX = {
    mybir.EngineType.PE: "PE_",
    mybir.EngineType.Activation: "Activation_",
    mybir.EngineType.DVE: "DVE_",
    mybir.EngineType.Pool: "Pool_",
}

# instruction types allowed to carry multiple waits (none on trn2 — every
# engine encoding holds a single sync-wait command)
_MULTIWAIT_OK = ()


def _fix_sync_waits(nc):
    """Enforce <=1 semaphore wait per engine instruction.

    The trn2 engine instruction encodings (S3_LW for matmul, S3D3_AC for
    activation, PSEUDO_DMA_DIRECT2D for HWDGE dma, ...) hold a single
    sync-wait command; walrus fails codegen with "Too many sync wait
    commands" otherwise. Two classes of redundant waits are dropped:

    1. own-engine waits: a wait on the instruction's own engine-completion
       semaphore. Compute queues execute and complete strictly in order
       (MATMULs are pc-monotone in start and end), so these are ordering
       no-ops emitted by Tile's bank-overlap guard.
    2. transitively-implied waits: wait (s2 >= v2) is dropped when another
       wait (s1 >= v1) of the same instruction implies it through the sem
       graph -- i.e. some instruction whose completion is counted in
       (s1 >= v1) itself waited on (s2 >= v2') with v2' >= v2 (closure
       computed over the whole program).

    Anything still >1 wait is a kernel-structure bug -- fail loudly at
    build time rather than at walrus codegen.
    """
    f = nc.m.functions[0]
    insts = [i for blk in f.blocks for i in blk.instructions]

    count = {}
    cover = {}
    stream_acc = {}

    def lookup(sem, val):
        """waits implied by 'sem has reached val'."""
        implied = {}
        for v_after, acc in cover.get(sem, []):
            if v_after <= val:
                implied.update(
                    {k: max(implied.get(k, -1), v) for k, v in acc.items()})
            else:
                break
        return implied

    # Per-compute-engine accumulated waits: engine queues execute and
    # complete strictly in order, so a sem update by instruction N implies
    # every wait carried by instructions 1..N of that engine — including
    # non-updating ones like InstLdweights (which carry the DMA wait for
    # the matmul that follows).
    eng_acc = {}

    for inst in insts:
        si = inst.sync_info
        waits = list(si.on_wait) if si is not None else []
        direct = {}
        for w in waits:
            direct[w.ant_name] = max(direct.get(w.ant_name, -1), w.wait_value)
            for k, v in lookup(w.ant_name, w.wait_value).items():
                direct[k] = max(direct.get(k, -1), v)
        if inst.engine in _OWN_SEM_PREFIX:
            acc_e = eng_acc.setdefault(inst.engine, {})
            for k, v in direct.items():
                acc_e[k] = max(acc_e.get(k, -1), v)
            direct = acc_e
        for u in (si.on_update if si is not None else []) or []:
            s = u.ant_name
            count[s] = count.get(s, 0) + u.update_value
            acc = stream_acc.setdefault(s, {})
            for k, v in direct.items():
                acc[k] = max(acc.get(k, -1), v)
            cover.setdefault(s, []).append((count[s], dict(acc)))

    for inst in insts:
        si = inst.sync_info
        if si is None or len(si.on_wait) <= 1:
            continue
        if isinstance(inst, _MULTIWAIT_OK):
            continue
        own = _OWN_SEM_PREFIX.get(inst.engine)
        kept = list(si.on_wait)
        if own is not None:
            kept = [w for w in kept if not w.ant_name.startswith(own)]
        if type(inst).__name__ == "InstDMACopy":
            # own-queue wait: a HWDGE queue processes its ring entries in
            # order, so a wait on the sem this DMA itself updates (its own
            # queue's completion sem) is a FIFO ordering no-op.
            own_q = {u.ant_name for u in (si.on_update or [])}
            kept = [w for w in kept if w.ant_name not in own_q]
        if len(kept) > 1:
            final = []
            for i, w in enumerate(kept):
                others = final + kept[i + 1:]
                if not any(
                    lookup(o.ant_name, o.wait_value).get(w.ant_name, -1) >= w.wait_value
                    for o in others
                ):
                    final.append(w)
            kept = final
        if len(kept) > 1:
            raise RuntimeError(
                f"{type(inst).__name__} {inst.name} still has {len(kept)} waits: "
                f"{[(w.ant_name, w.wait_value) for w in si.on_wait]}"
            )
        inst.sync_info = mybir.SyncInfo(on_wait=kept, on_update=si.on_update)


def _emit(nc: bass.Bass, fix_waits=True):
    docp = nc.dram_tensor("docp", [128, BPC * HT * 512], BF16, kind="ExternalInput").ap()
    cons16 = nc.dram_tensor("cons16", [128, CONS16_COLS], BF16, kind="ExternalInput").ap()
    aux32 = nc.dram_tensor("aux32", [128, AUX_COLS], F32, kind="ExternalInput").ap()
    # raw scores + importance column per batch: [33, 512] each. The masking
    # (relu/stopword/doc-mask) and the max over doc positions run on the
    # host, which keeps the device at ONE score matmul per batch.
    mout = nc.dram_tensor("mout", [EW, BPC * LD], F32, kind="ExternalOutput").ap()

    with tile.TileContext(nc) as tc, ExitStack() as ctx:
        singles = ctx.enter_context(tc.tile_pool(name="singles", bufs=1))
        xp = ctx.enter_context(tc.tile_pool(name="xp", bufs=5))
        dp = ctx.enter_context(tc.tile_pool(name="dp", bufs=3))
        pw = ctx.enter_context(tc.tile_pool(name="pw", bufs=1, space="PSUM"))
        pq = ctx.enter_context(tc.tile_pool(name="pq", bufs=1, space="PSUM"))
        pd = ctx.enter_context(tc.tile_pool(name="pd", bufs=2, space="PSUM"))
        ps = ctx.enter_context(tc.tile_pool(name="ps", bufs=2, space="PSUM"))

        cons_sb = singles.tile([128, CONS16_COLS], BF16)
        aux_sb = singles.tile([128, AUX_COLS], F32)
        qvw_sb = singles.tile([128, BPC * EW], BF16)
        mo_sb = singles.tile([EW, BPC * LD], F32)
        touch_a = singles.tile([128, 1], F32)

        # PE p-state warmup: the first ~9us of the kernel are preamble + DMA
        # issue with an idle PE, which leaves the tensor engine cold right
        # when the doc stream arrives. Run dummy matmuls on uninitialized
        # SBUF garbage (no DMA wait, results discarded) so PE enters the
        # stream at full speed. psum_qv reuses the bank afterwards (PE-own
        # ordering).
        N_WARM = 16
        garbage = singles.tile([128, LD], BF16)
        nc.vector.memset(garbage[:], 1.0)
        dummy = pw.tile([128, LD], F32, name="dummy")
        for i in range(N_WARM):
            nc.tensor.matmul(dummy[:], garbage[:, 0:128], garbage[:],
                             start=True, stop=True)

        nc.sync.dma_start(out=cons_sb[:], in_=cons16)
        nc.sync.dma_start(out=aux_sb[:], in_=aux32)
        # pre-observe the aux DMA lane on ACT so later consumers don't need
        # a second sync wait on their instruction.
        nc.scalar.copy(touch_a[:], aux_sb[:, 0:1])

        w_sb = cons_sb[:, CONS_W:CONS_W + HT * 128]
        qt_sb = cons_sb[:, CONS_QT:CONS_QT + HT * 512]
        bcomp_ap = aux_sb[:, A_BCOMP:A_BCOMP + 1]
        bstop_ap = aux_sb[:, A_BSTOP:A_BSTOP + 1]

        # q_vecs^T (bf16, bias folded) for all 16 batches, interleaved with a
        # w_stop column per batch: qvw[:, b*33:(b+1)*33] = [q_vecs^T(b) | w_stop]
        qvw3 = qvw_sb[:].rearrange("p (b e) -> p b e", e=EW)
        psum_qv = pq.tile([128, BPC * LQ], F32)
        for ht in range(HT):
            nc.tensor.matmul(
                psum_qv[:],
                w_sb[:, ht * 128:(ht + 1) * 128],
                qt_sb[:, ht * 512:(ht + 1) * 512],
                start=(ht == 0),
                stop=(ht == HT - 1),
            )
        nc.scalar.activation(
            qvw3[:, :, 0:LQ],
            psum_qv[:].rearrange("p (b q) -> p b q", q=LQ),
            AF.Identity, bias=bcomp_ap, scale=1.0,
        )
        nc.scalar.copy(qvw3[:, :, LQ:EW],
                       aux_sb[:, A_WST16:A_WST16 + BPC].rearrange("p (b o) -> p b o", o=1))



        # doc chunk DMAs are emitted one chunk AHEAD of their compute (and
        # the first two before any compute) so the per-chunk mout DMA never
        # head-of-line-blocks the next doc chunk on the queue.
        nchunks = len(CHUNK_SIZES)
        starts = np.cumsum([0] + CHUNK_SIZES).tolist()
        xts = [None] * nchunks

        def dma_chunk(k):
            nb = CHUNK_SIZES[k]
            xts[k] = xp.tile([128, CHUNK * HT * 512], BF16, tag="xt", name="xt")
            lo = starts[k] * HT * 512
            nc.sync.dma_start(out=xts[k][:, 0:nb * HT * 512],
                              in_=docp[:, lo:lo + nb * HT * 512])

        dma_chunk(0)
        dma_chunk(1)

        dma_chunk(2)
        for k in range(nchunks):
            if k + 3 < nchunks:
                dma_chunk(k + 3)
            xt = xts[k]
            if k >= 1:
                # p-state filler: absorb the chunk-boundary DMA wait with
                # dummy matmuls so the tensor engine never idles (an idle
                # gap resets PE to the half-speed pipeline state for ~3us).
                for _ in range(2):
                    nc.tensor.matmul(dummy[:], garbage[:, 0:128], garbage[:],
                                     start=True, stop=True)

            for bi in range(CHUNK_SIZES[k]):
                gb = starts[k] + bi
                # d_tok^T [c, l] via 6 accumulating matmuls (matmul output is
                # capped at one PSUM bank = 512 fp32, so one batch per group)
                psum_d = pd.tile([128, LD], F32, tag="pd", name="psum_d")
                for ht in range(HT):
                    o = (bi * HT + ht) * 512
                    nc.tensor.matmul(
                        psum_d[:],
                        w_sb[:, ht * 128:(ht + 1) * 128],
                        xt[:, o:o + 512],
                        start=(ht == 0),
                        stop=(ht == HT - 1),
                    )
                d_sb = dp.tile([128, LD], BF16)
                nc.scalar.activation(d_sb[:], psum_d[:],
                                     AF.Identity, bias=bcomp_ap, scale=1.0)

                # raw scores + importance column in ONE matmul:
                # psum_s[q', l] = qvw(b)^T @ d_sb  -> [33, 512]
                psum_s = ps.tile([EW, LD], F32)
                nc.tensor.matmul(
                    psum_s[:],
                    qvw_sb[:, gb * EW:(gb + 1) * EW],
                    d_sb[:],
                    start=True,
                    stop=True,
                )
                nc.scalar.copy(mo_sb[:, gb * LD:(gb + 1) * LD], psum_s[:])

        nc.sync.dma_start(out=mout, in_=mo_sb[:])
    if fix_waits:
        _fix_sync_waits(nc)
    return nc


def _get_nc(fix_waits=True):
    key = ("nc", fix_waits)
    if key not in _CACHE:
        nc = bass.Bass("TRN2", target_bir_lowering=False, debug=False,
                       num_devices=NCORES)
        _emit(nc, fix_waits=fix_waits)
        _CACHE[key] = nc
    return _CACHE[key]


def make_in_maps(query_hidden, doc_hidden, query_mask, doc_mask,
                 W_comp, b_comp, w_stop, b_stop, score_merger):
    """Host-side shard + relayout + bf16 cast. Returns list of 8 in_maps."""
    q = np.asarray(query_hidden, dtype=np.float32).astype(NP_BF16)
    d = np.asarray(doc_hidden, dtype=np.float32).astype(NP_BF16)
    W = np.asarray(W_comp, dtype=np.float32).astype(NP_BF16)

    # doc: (core, b, l, ht, hp) -> (core, hp, b, ht, l)
    docp = np.ascontiguousarray(
        d.reshape(NCORES, BPC, LD, HT, 128).transpose(0, 4, 1, 3, 2)
    ).reshape(NCORES, 128, BPC * HT * 512)

    # query: (core, b, q, ht, hp) -> (core, hp, ht, b, q)
    qtp = np.ascontiguousarray(
        q.reshape(NCORES, BPC, LQ, HT, 128).transpose(0, 4, 3, 1, 2)
    ).reshape(NCORES, 128, HT * 512)

    # W: (ht, hp, c) -> (hp, ht, c)
    wp = np.ascontiguousarray(W.reshape(HT, 128, C).transpose(1, 0, 2)).reshape(128, HT * 128)

    cons = np.zeros((NCORES, 128, CONS16_COLS), dtype=NP_BF16)
    cons[:, :, CONS_W:CONS_W + HT * 128] = wp[None]
    cons[:, :, CONS_QT:CONS_QT + HT * 512] = qtp

    dm_f = np.asarray(doc_mask).astype(np.float32)
    # (core, b, kt, kp) -> (core, kp, b, kt)
    dmk = np.ascontiguousarray(
        dm_f.reshape(NCORES, BPC, KT, 128).transpose(0, 3, 1, 2)
    ).reshape(NCORES, 128, BPC * KT)
    offk = (dmk - 1.0) * 1000.0

    aux = np.zeros((NCORES, 128, AUX_COLS), dtype=np.float32)
    aux[:, :, A_BCOMP] = np.asarray(b_comp, dtype=np.float32)[None, :]
    aux[:, :, A_BSTOP] = np.float32(np.asarray(b_stop, dtype=np.float32)[0])
    aux[:, :, A_WST16:A_WST16 + BPC] = np.asarray(w_stop, dtype=np.float32)[None, :, 0:1]
    aux[:, :, A_DMK:A_DMK + BPC * KT] = dmk
    aux[:, :, A_OFFK:A_OFFK + BPC * KT] = offk

    in_maps = []
    for c in range(NCORES):
        in_maps.append({
            "docp": np.ascontiguousarray(docp[c]),
            "cons16": np.ascontiguousarray(cons[c]),
            "aux32": np.ascontiguousarray(aux[c]),
        })
    return in_maps


def host_epilogue(mout_list, query_hidden, doc_hidden, query_mask, doc_mask,
                  b_stop, score_merger):
    """mout_list: list of 8 [EW, BPC*LD] arrays (raw scores + imp column)."""
    # s_raw[b, q', l]: rows 0..31 are q_vec·d_tok scores, row 32 is d_tok·w_stop
    s_raw = np.concatenate(
        [m.reshape(EW, BPC, LD).transpose(1, 0, 2) for m in mout_list], axis=0
    ).astype(np.float32)  # [B, EW, LD]
    s = s_raw[:, 0:LQ, :]                       # [B, LQ, LD]
    ic = s_raw[:, LQ, :]                        # [B, LD]
    bs = np.float32(np.asarray(b_stop, dtype=np.float32)[0])
    imp = np.maximum(ic + bs, np.float32(0.0))  # [B, LD]
    dm = np.asarray(doc_mask) != 0              # [B, LD]
    sm = s * (imp * dm)[:, None, :]
    sm = np.where(dm[:, None, :], sm, np.float32(-1000.0))
    term = sm.max(axis=-1).astype(np.float32)   # [B, LQ]
    qm = np.asarray(query_mask).astype(bool)
    term_score = np.where(qm, term, np.float32(0.0)).astype(np.float32).sum(axis=-1, dtype=np.float32)

    q_cls = np.asarray(query_hidden, dtype=np.float32)[:, 0, :]
    d_cls = np.asarray(doc_hidden, dtype=np.float32)[:, 0, :]
    cls_score = np.sum(q_cls * d_cls, axis=-1, dtype=np.float32)

    sm = np.float32(np.asarray(score_merger, dtype=np.float32)[0])
    w = np.float32(1.0) / (np.float32(1.0) + np.exp(-sm, dtype=np.float32))
    cls_out = (cls_score * w).astype(np.float32)
    term_out = (term_score * (np.float32(1.0) - w)).astype(np.float32)
    score = (cls_out + term_out).astype(np.float32)
    return score, cls_out, term_out


def kernel(query_hidden, doc_hidden, query_mask, doc_mask,
           W_comp, b_comp, w_stop, b_stop, score_merger):
    nc = _get_nc()
    in_maps = make_in_maps(query_hidden, doc_hidden, query_mask, doc_mask,
                           W_comp, b_comp, w_stop, b_stop, score_merger)
    res = bass_utils.run_bass_kernel_spmd(nc, in_maps, core_ids=list(range(NCORES)))
    mout_list = [res.results[c]["mout"] for c in range(NCORES)]
    return host_epilogue(mout_list, query_hidden, doc_hidden, query_mask,
                         doc_mask, b_stop, score_merger)


# revision 34
# speedup vs baseline: 1.0446x; 1.0446x over previous
"""ColBERTer forward as a Trainium2 Bass/Tile kernel, data-parallel over 8 cores.

Problem shapes (hardcoded): B=128, LQ=32, LD=512, H=768, C=128.

Strategy (v8: query-side folding, bf16 datapath)
------------------------------------------------
Pure data parallel: batch dim sharded 16-per-core across 8 NeuronCores.

The compressor is algebraically folded into the query side:

    s[q, l] = q_vec[q] . (W^T h_l + b_comp)
            = (W q_vec[q]) . h_l  +  q_vec[q] . b_comp

so the HOST precomputes (cheap BLAS, ~1 GFLOP total):
    qv   = query_hidden @ W + b_comp            [B, LQ, C]
    z    = qv @ W^T   (+ w_stop column: W@w_stop)  -> [B, EW=33, H]
    bq   = qv . b_comp (per q), bw = b_comp . w_stop

and the DEVICE streams doc_hidden (bf16, the memory-roofline traffic)
through ONE accumulating matmul chain per batch:

    psum_s[q', l] = sum_ht z_tile[ht]^T @ doc_tile[ht]     (6 matmuls)

giving raw scores + the importance column in a [33, 512] psum, copied to
SBUF as bf16 and DMA'd out. Stopword relu, doc/query masking, the max
over doc positions, cls score, and the sigmoid merge all run on the host
in fp32 (numerically identical structure to the reference).

The 33-column stationary makes the per-matmul weight load ~4x cheaper
than a 128-wide compressor tile, and there are 6 matmuls per batch
instead of 10 (~110ns fixed cost per PE instruction on trn2).
"""

import numpy as np
from contextlib import ExitStack

import concourse.bass as bass
import concourse.tile as tile
from concourse import mybir
from concourse import bass_utils

F32 = mybir.dt.float32
BF16 = mybir.dt.bfloat16
NP_BF16 = mybir.dt.np(BF16)
AF = mybir.ActivationFunctionType
ALU = mybir.AluOpType

B, LQ, LD, H, C = 128, 32, 512, 768, 128
NCORES = 8
BPC = B // NCORES       # 16 batches per core
HT = H // 128           # 6 h-tiles
KT = LD // 128          # 4 doc-token tiles
CHUNK = 2               # batches per doc DMA chunk
CHUNK_SIZES = [2] * 8
EW = LQ + 1             # 32 score rows + 1 importance row

_CACHE = {}


# engine -> its own semaphore-name prefix (strict-FIFO compute queues only;
# a wait on the engine's OWN completion sem is an ordering no-op on these).
_OWN_SEM_PREFIX = {
    mybir.EngineType.PE: "PE_",
    mybir.EngineType.Activation: "Activation_",
    mybir.EngineType.DVE: "DVE_",
    mybir.EngineType.Pool: "Pool_",
}

# instruction types allowed to carry multiple waits (none on trn2 — every
# engine encoding holds a single sync-wait command)
_MULTIWAIT_OK = ()


def _fix_sync_waits(nc):
    """Enforce <=1 semaphore wait per engine instruction.

    The trn2 engine instruction encodings (S3_LW for matmul, S3D3_AC for
    activation, PSEUDO_DMA_DIRECT2D for HWDGE dma, ...) hold a single
    sync-wait command; walrus fails codegen with "Too many sync wait
    commands" otherwise. Three classes of redundant waits are dropped:

    1. own-engine waits: a wait on the instruction's own engine-completion
       semaphore. Compute queues execute and complete strictly in order,
       so these are ordering no-ops emitted by Tile's bank-overlap guard.
    2. own-queue DMA waits: a HWDGE queue processes its ring entries in
       order, so a DMA waiting on the sem it itself updates is a FIFO
       ordering no-op.
    3. transitively-implied waits: wait (s2 >= v2) is dropped when another
       wait (s1 >= v1) of the same instruction implies it through the sem
       graph (closure over the whole program, including waits carried by
       non-updating instructions like InstLdweights via their engine's
       FIFO order).

    Anything still >1 wait is a kernel-structure bug -- fail loudly at
    build time rather than at walrus codegen.
    """
    f = nc.m.functions[0]
    insts = [i for blk in f.blocks for i in blk.instructions]

    count = {}
    cover = {}
    stream_acc = {}
    eng_acc = {}

    def lookup(sem, val):
        implied = {}
        for v_after, acc in cover.get(sem, []):
            if v_after <= val:
                implied.update(
                    {k: max(implied.get(k, -1), v) for k, v in acc.items()})
            else:
                break
        return implied

    for inst in insts:
        si = inst.sync_info
        waits = list(si.on_wait) if si is not None else []
        direct = {}
        for w in waits:
            direct[w.ant_name] = max(direct.get(w.ant_name, -1), w.wait_value)
            for k, v in lookup(w.ant_name, w.wait_value).items():
                direct[k] = max(direct.get(k, -1), v)
        if inst.engine in _OWN_SEM_PREFIX:
            acc_e = eng_acc.setdefault(inst.engine, {})
            for k, v in direct.items():
                acc_e[k] = max(acc_e.get(k, -1), v)
            direct = acc_e
        for u in (si.on_update if si is not None else []) or []:
            s = u.ant_name
            count[s] = count.get(s, 0) + u.update_value
            acc = stream_acc.setdefault(s, {})
            for k, v in direct.items():
                acc[k] = max(acc.get(k, -1), v)
            cover.setdefault(s, []).append((count[s], dict(acc)))

    # Pass 2 re-walks in order, tracking per-engine accumulated waits a
    # second time so that a wait already guaranteed by an EARLIER
    # instruction of the same (FIFO) engine can be dropped (rule 4).
    eng_acc2 = {}
    for inst in insts:
        si = inst.sync_info
        waits = list(si.on_wait) if si is not None else []
        fifo_sat = eng_acc2.get(inst.engine, {})
        process = (si is not None and len(si.on_wait) > 1
                   and not isinstance(inst, _MULTIWAIT_OK))
        if not process:
            if inst.engine in _OWN_SEM_PREFIX and waits:
                acc_e = eng_acc2.setdefault(inst.engine, {})
                for w in waits:
                    acc_e[w.ant_name] = max(acc_e.get(w.ant_name, -1), w.wait_value)
                    for k, v in lookup(w.ant_name, w.wait_value).items():
                        acc_e[k] = max(acc_e.get(k, -1), v)
            continue
        own = _OWN_SEM_PREFIX.get(inst.engine)
        kept = list(si.on_wait)
        if own is not None:
            kept = [w for w in kept if not w.ant_name.startswith(own)]
            # rule 4: engine-FIFO-satisfied waits
            kept = [w for w in kept
                    if fifo_sat.get(w.ant_name, -1) < w.wait_value]
        if type(inst).__name__ == "InstDMACopy":
            own_q = {u.ant_name for u in (si.on_update or [])}
            kept = [w for w in kept if w.ant_name not in own_q]
        if inst.engine in _OWN_SEM_PREFIX:
            acc_e = eng_acc2.setdefault(inst.engine, {})
            for w in waits:
                acc_e[w.ant_name] = max(acc_e.get(w.ant_name, -1), w.wait_value)
                for k, v in lookup(w.ant_name, w.wait_value).items():
                    acc_e[k] = max(acc_e.get(k, -1), v)
        if len(kept) > 1:
            final = []
            for i, w in enumerate(kept):
                others = final + kept[i + 1:]
                if not any(
                    lookup(o.ant_name, o.wait_value).get(w.ant_name, -1) >= w.wait_value
                    for o in others
                ):
                    final.append(w)
            kept = final
        if len(kept) > 1:
            raise RuntimeError(
                f"{type(inst).__name__} {inst.name} still has {len(kept)} waits: "
                f"{[(w.ant_name, w.wait_value) for w in si.on_wait]}"
            )
        inst.sync_info = mybir.SyncInfo(on_wait=kept, on_update=si.on_update)


def _emit(nc: bass.Bass, fix_waits=True):
    # zp[p, ht*BPC*EW + b*EW + q'] = z[b, q', ht*128+p]  (bf16)
    zp = nc.dram_tensor("zp", [128, HT * BPC * EW], BF16, kind="ExternalInput").ap()
    # docp[p, (b*HT+ht)*512 + l] = doc_hidden[core*BPC+b, l, ht*128+p]  (bf16)
    docp = nc.dram_tensor("docp", [128, BPC * HT * 512], BF16, kind="ExternalInput").ap()
    # raw scores + importance row per batch, bf16: [33, b*512+l]
    mout = nc.dram_tensor("mout", [EW, BPC * LD], BF16, kind="ExternalOutput").ap()

    with tile.TileContext(nc) as tc, ExitStack() as ctx:
        singles = ctx.enter_context(tc.tile_pool(name="singles", bufs=1))
        xp = ctx.enter_context(tc.tile_pool(name="xp", bufs=5))
        pw = ctx.enter_context(tc.tile_pool(name="pw", bufs=1, space="PSUM"))
        ps = ctx.enter_context(tc.tile_pool(name="ps", bufs=4, space="PSUM"))

        z_sb = singles.tile([128, HT * BPC * EW], BF16)
        mo_sb = singles.tile([EW, BPC * LD], BF16)
        garbage = singles.tile([128, LD], BF16)

        # PE p-state warmup: the first ~9us are framework preamble + DMA
        # issue with an idle PE, which leaves the tensor engine in its
        # slow pipeline state right when the doc stream arrives. Chew on
        # memset garbage (no DMA dependency, results discarded).
        N_WARM = 10
        nc.vector.memset(garbage[:], 1.0)
        dummy = pw.tile([128, LD], F32, name="dummy")
        for i in range(N_WARM):
            nc.tensor.matmul(dummy[:], garbage[:, 0:128], garbage[:],
                             start=True, stop=True)

        nc.sync.dma_start(out=z_sb[:], in_=zp)

        nchunks = len(CHUNK_SIZES)
        starts = np.cumsum([0] + CHUNK_SIZES).tolist()
        xts = [None] * nchunks

        def dma_chunk(k):
            nb = CHUNK_SIZES[k]
            xts[k] = xp.tile([128, CHUNK * HT * 512], BF16, tag="xt", name="xt")
            lo = starts[k] * HT * 512
            nc.sync.dma_start(out=xts[k][:, 0:nb * HT * 512],
                              in_=docp[:, lo:lo + nb * HT * 512])

        dma_chunk(0)
        dma_chunk(1)
        dma_chunk(2)
        for k in range(nchunks):
            if k + 3 < nchunks:
                dma_chunk(k + 3)
            xt = xts[k]
            if k >= 2:
                # PE observer: a 1x1 matmul reading the last mo_sb column of
                # chunk k-2. It advances PE's observed ACT clock past that
                # chunk's psum_s->SBUF copies, so this chunk's matmuls carry
                # only their DMA wait when reusing a psum_s bank (the trn2
                # matmul encoding holds a single sync-wait command).
                oc = starts[k - 1] * LD - 1
                nc.tensor.matmul(dummy[0:1, 0:1], mo_sb[0:1, oc:oc + 1],
                                 mo_sb[0:1, oc:oc + 1], start=True, stop=True)
            for bi in range(CHUNK_SIZES[k]):
                gb = starts[k] + bi
                # raw scores + importance row via 6 accumulating matmuls:
                # psum_s[q', l] = sum_ht z_tile[b,ht]^T @ doc_tile[b,ht]
                psum_s = ps.tile([EW, LD], F32)
                for ht in range(HT):
                    o = (bi * HT + ht) * 512
                    zo = (ht * BPC + gb) * EW
                    nc.tensor.matmul(
                        psum_s[:],
                        z_sb[:, zo:zo + EW],
                        xt[:, o:o + 512],
                        start=(ht == 0),
                        stop=(ht == HT - 1),
                    )
                nc.scalar.copy(mo_sb[:, gb * LD:(gb + 1) * LD], psum_s[:])

        nc.sync.dma_start(out=mout, in_=mo_sb[:])
    if fix_waits:
        _fix_sync_waits(nc)
    return nc


def _get_nc(fix_waits=True):
    key = ("nc", fix_waits)
    if key not in _CACHE:
        nc = bass.Bass("TRN2", target_bir_lowering=False, debug=False,
                       num_devices=NCORES)
        _emit(nc, fix_waits=fix_waits)
        _CACHE[key] = nc
    return _CACHE[key]


def make_in_maps(query_hidden, doc_hidden, query_mask, doc_mask,
                 W_comp, b_comp, w_stop, b_stop, score_merger):
    """Host-side shard + relayout + query-side folding. Returns 8 in_maps."""
    q = np.asarray(query_hidden, dtype=np.float32)
    d = np.asarray(doc_hidden, dtype=np.float32).astype(NP_BF16)
    W = np.asarray(W_comp, dtype=np.float32)
    bc = np.asarray(b_comp, dtype=np.float32)
    ws = np.asarray(w_stop, dtype=np.float32)[:, 0]

    # qv = q @ W + b  (fp32, exact query-side math)
    qv = q.reshape(B * LQ, H) @ W + bc                     # [B*LQ, C]
    # z[b, q', h]: scores rows = qv @ W^T, importance row = W @ w_stop
    zq = (qv @ W.T).reshape(B, LQ, H)                      # [B, LQ, H]
    zw = W @ ws                                            # [H]
    z = np.concatenate([zq, np.broadcast_to(zw, (B, 1, H))], axis=1)  # [B, EW, H]
    z16 = z.astype(NP_BF16)

    # z: (core, b, e, ht, hp) -> (core, hp, ht, b, e)
    zp = np.ascontiguousarray(
        z16.reshape(NCORES, BPC, EW, HT, 128).transpose(0, 4, 3, 1, 2)
    ).reshape(NCORES, 128, HT * BPC * EW)

    # doc: (core, b, l, ht, hp) -> (core, hp, b, ht, l)
    docp = np.ascontiguousarray(
        d.reshape(NCORES, BPC, LD, HT, 128).transpose(0, 4, 1, 3, 2)
    ).reshape(NCORES, 128, BPC * HT * 512)

    in_maps = []
    for c in range(NCORES):
        in_maps.append({
            "zp": np.ascontiguousarray(zp[c]),
            "docp": np.ascontiguousarray(docp[c]),
        })
    # host-side constants for the epilogue
    bq = (qv @ bc).reshape(B, LQ).astype(np.float32)       # qv . b_comp
    bw = np.float32(bc @ ws)                               # b_comp . w_stop
    return in_maps, bq, bw


def host_epilogue(mout_list, bq, bw, query_hidden, doc_hidden, query_mask,
                  doc_mask, b_stop, score_merger):
    """mout_list: list of 8 [EW, BPC*LD] bf16 arrays (raw scores + imp row)."""
    s_raw = np.concatenate(
        [np.asarray(m, dtype=np.float32).reshape(EW, BPC, LD).transpose(1, 0, 2)
         for m in mout_list], axis=0
    )                                            # [B, EW, LD]
    s = s_raw[:, 0:LQ, :] + bq[:, :, None]       # q_vec . d_tok
    ic = s_raw[:, LQ, :] + bw                    # d_tok . w_stop
    bs = np.float32(np.asarray(b_stop, dtype=np.float32)[0])
    imp = np.maximum(ic + bs, np.float32(0.0))   # [B, LD]
    dm = np.asarray(doc_mask) != 0               # [B, LD]
    sm = s * (imp * dm)[:, None, :]
    sm = np.where(dm[:, None, :], sm, np.float32(-1000.0))
    term = sm.max(axis=-1).astype(np.float32)    # [B, LQ]
    qm = np.asarray(query_mask).astype(bool)
    term_score = np.where(qm, term, np.float32(0.0)).astype(np.float32).sum(axis=-1, dtype=np.float32)

    q_cls = np.asarray(query_hidden, dtype=np.float32)[:, 0, :]
    d_cls = np.asarray(doc_hidden, dtype=np.float32)[:, 0, :]
    cls_score = np.sum(q_cls * d_cls, axis=-1, dtype=np.float32)

    sm_ = np.float32(np.asarray(score_merger, dtype=np.float32)[0])
    w = np.float32(1.0) / (np.float32(1.0) + np.exp(-sm_, dtype=np.float32))
    cls_out = (cls_score * w).astype(np.float32)
    term_out = (term_score * (np.float32(1.0) - w)).astype(np.float32)
    score = (cls_out + term_out).astype(np.float32)
    return score, cls_out, term_out


def kernel(query_hidden, doc_hidden, query_mask, doc_mask,
           W_comp, b_comp, w_stop, b_stop, score_merger):
    nc = _get_nc()
    in_maps, bq, bw = make_in_maps(query_hidden, doc_hidden, query_mask, doc_mask,
                                   W_comp, b_comp, w_stop, b_stop, score_merger)
    res = bass_utils.run_bass_kernel_spmd(nc, in_maps, core_ids=list(range(NCORES)))
    mout_list = [res.results[c]["mout"] for c in range(NCORES)]
    return host_epilogue(mout_list, bq, bw, query_hidden, doc_hidden,
                         query_mask, doc_mask, b_stop, score_merger)
